# revision 7
# baseline (speedup 1.0000x reference)
"""nn_Decoder Trainium2 kernel: 8-core data-parallel MHA decoder + CRF.

Device (per core, 4 examples): x -> MHA(2 heads) -> ReLU -> emissions [S,T]
and entity logits [S,2], all fp32 on the PE/ACT/DVE engines.
Host: tiny O(B*S*T^2) CRF numerator/normalizer/viterbi + log_softmax from
device-computed emissions (exactly mirrors the reference math in fp32).
"""
import sys

for _p in ("/opt/trn_rl_repo",):
    if _p not in sys.path:
        sys.path.append(_p)

import numpy as np
import concourse.bass as bass
import concourse.mybir as mybir
import concourse.tile as tile
import concourse.bacc as bacc
from concourse import masks
from concourse.bass_utils import run_bass_kernel_spmd

F32 = mybir.dt.float32
B, S, E, T, H = 32, 512, 512, 24, 2
HD = E // H            # 256 head dim
NC = 8                 # cores
BS = B // NC           # 4 examples per core
EC = E // 128          # 4 chunks of the embedding dim
AF = mybir.ActivationFunctionType

_cached = {}


def _build():
    nc = bacc.Bacc(None, target_bir_lowering=False, debug=False)
    WCOL = 3 * E + E + T + 2  # 2074 packed weight columns per e-chunk
    x_d = nc.declare_dram_parameter("x", [BS, S, E], F32, isOutput=False)
    wpack_d = nc.declare_dram_parameter("wpack", [EC, 128, WCOL], F32, isOutput=False)
    bpack_d = nc.declare_dram_parameter("bpack", [128, 16], F32, isOutput=False)
    sbias_d = nc.declare_dram_parameter("sbias", [32, 2], F32, isOutput=False)
    fcT_d = nc.declare_dram_parameter("fcT", [BS, T, S], F32, isOutput=True)
    segT_d = nc.declare_dram_parameter("segT", [BS, 2, S], F32, isOutput=True)

    with tile.TileContext(nc) as tc:
        with (
            tc.tile_pool(name="wpool", bufs=1) as wpool,
            tc.tile_pool(name="apool", bufs=1) as apool,
            tc.tile_pool(name="spool", bufs=2) as spool,
            tc.tile_pool(name="psmm", bufs=4, space="PSUM") as psmm,
            tc.tile_pool(name="pstp", bufs=2, space="PSUM") as pstp,
        ):
            ident = wpool.tile([128, 128], F32, tag="ident")
            masks.make_identity(nc, ident[:])

            wsb = wpool.tile([128, EC, WCOL], F32, tag="wsb")
            bpack_sb = wpool.tile([128, 16], F32, tag="bpack")
            sbias_sb = wpool.tile([32, 2], F32, tag="sbias")
            nc.sync.dma_start(wsb[:], wpack_d[:].rearrange("c p f -> p c f"))
            nc.sync.dma_start(bpack_sb[:], bpack_d[:])
            nc.sync.dma_start(sbias_sb[:], sbias_d[:])

            for i in range(BS):
                # ---- load x and transpose to xT [e, s] ----
                xT = apool.tile([128, EC, S], F32, tag="xT")
                xn = spool.tile([128, 4, E], F32, tag="xn")
                nc.sync.dma_start(xn[:], x_d[i].rearrange("(sc p) e -> p sc e", p=128))
                for sc in range(4):
                    for ec in range(EC):
                        tp = pstp.tile([128, 128], F32, tag="tp")
                        nc.tensor.matmul(
                            tp[:], xn[:, sc, ec * 128 : (ec + 1) * 128], ident[:],
                            is_transpose=True,
                        )
                        nc.vector.tensor_copy(
                            xT[:, ec, sc * 128 : (sc + 1) * 128], tp[:]
                        )
                # ---- qkvT = Win @ x.T + bin  [e', s] (12 chunks) ----
                qkvT = apool.tile([128, 12, S], F32, tag="qkvT")
                for j in range(12):
                    ps = psmm.tile([128, S], F32, tag="mm")
                    for ec in range(EC):
                        nc.tensor.matmul(
                            ps[:],
                            wsb[:, ec, j * 128 : (j + 1) * 128],
                            xT[:, ec, :],
                            start=(ec == 0), stop=(ec == EC - 1),
                        )
                    nc.scalar.activation(
                        qkvT[:, j, :], ps[:], AF.Identity, bias=bpack_sb[:, j : j + 1]
                    )
                # ---- v natural [s, d]: transpose vT chunks (j = 8..11) ----
                vnat = apool.tile([128, 4, 4, 128], F32, tag="vnat")  # [s, sc, vc, d]
                for vc in range(4):
                    for sc in range(4):
                        tp = pstp.tile([128, 128], F32, tag="tp")
                        nc.tensor.matmul(
                            tp[:], qkvT[:, 8 + vc, sc * 128 : (sc + 1) * 128],
                            ident[:], is_transpose=True,
                        )
                        nc.vector.tensor_copy(vnat[:, sc, vc, :], tp[:])
                # ---- attention per head ----
                attnT = apool.tile([128, H, 4, 4, 128], F32, tag="attnT")  # ks,[h,kc,qsc,qs]
                for h in range(H):
                    for qsc in range(4):
                        sc_ps = psmm.tile([128, S], F32, tag="mm")
                        for dc in range(2):
                            j = h * 2 + dc
                            nc.tensor.matmul(
                                sc_ps[:],
                                qkvT[:, j, qsc * 128 : (qsc + 1) * 128],
                                qkvT[:, 4 + j, :],
                                start=(dc == 0), stop=(dc == 1),
                            )
                        mx = spool.tile([128, 1], F32, tag="mx")
                        nc.vector.reduce_max(
                            mx[:], sc_ps[:], axis=mybir.AxisListType.X, negate=True
                        )
                        mxs = spool.tile([128, 1], F32, tag="mxs")
                        nc.vector.tensor_scalar_mul(mxs[:], mx[:], 1.0 / 16.0)
                        attn = spool.tile([128, S], F32, tag="attn")
                        nc.scalar.activation(
                            attn[:], sc_ps[:], AF.Exp, bias=mxs[:], scale=1.0 / 16.0
                        )
                        sm = spool.tile([128, 1], F32, tag="sm")
                        nc.vector.reduce_sum(sm[:], attn[:], axis=mybir.AxisListType.X)
                        rs = spool.tile([128, 1], F32, tag="rs")
                        nc.vector.reciprocal(rs[:], sm[:])
                        nc.vector.tensor_scalar_mul(attn[:], attn[:], rs[:])
                        for kc in range(4):
                            tp = pstp.tile([128, 128], F32, tag="tp")
                            nc.tensor.matmul(
                                tp[:], attn[:, kc * 128 : (kc + 1) * 128],
                                ident[:], is_transpose=True,
                            )
                            nc.vector.tensor_copy(attnT[:, h, kc, qsc, :], tp[:])
                # ---- attn_outT [e', s] = v.T @ attn.T ----
                aoT = apool.tile([128, EC, S], F32, tag="aoT")
                for h in range(H):
                    for dc in range(2):
                        vc = h * 2 + dc
                        ao_ps = psmm.tile([128, S], F32, tag="mm")
                        for qsc in range(4):
                            for kc in range(4):
                                nc.tensor.matmul(
                                    ao_ps[:, qsc * 128 : (qsc + 1) * 128],
                                    vnat[:, kc, vc, :],
                                    attnT[:, h, kc, qsc, :],
                                    start=(kc == 0), stop=(kc == 3),
                                )
                        nc.scalar.activation(aoT[:, vc, :], ao_ps[:], AF.Identity, bias=0.0)
                # ---- decT = relu(Wout @ attn_out.T + bout)  [e', s] ----
                decT = apool.tile([128, EC, S], F32, tag="decT")
                for jc in range(EC):
                    yt = psmm.tile([128, S], F32, tag="mm")
                    for ec in range(EC):
                        nc.tensor.matmul(
                            yt[:],
                            wsb[:, ec, 3 * E + jc * 128 : 3 * E + (jc + 1) * 128],
                            aoT[:, ec, :],
                            start=(ec == 0), stop=(ec == EC - 1),
                        )
                    nc.scalar.activation(
                        decT[:, jc, :], yt[:], AF.Relu, bias=bpack_sb[:, 12 + jc : 13 + jc]
                    )
                # ---- heads: fcT [T, s], segT [2, s] ----
                fc_ps = psmm.tile([T, S], F32, tag="mm")
                for ec in range(EC):
                    nc.tensor.matmul(
                        fc_ps[:], wsb[:, ec, 4 * E : 4 * E + T], decT[:, ec, :],
                        start=(ec == 0), stop=(ec == EC - 1),
                    )
                fcT_sb = spool.tile([T, S], F32, tag="fcT")
                nc.scalar.activation(fcT_sb[:], fc_ps[:], AF.Identity, bias=sbias_sb[0:T, 0:1])
                nc.sync.dma_start(fcT_d[i], fcT_sb[:])

                seg_ps = psmm.tile([2, S], F32, tag="mm")
                for ec in range(EC):
                    nc.tensor.matmul(
                        seg_ps[:], wsb[:, ec, 4 * E + T : 4 * E + T + 2], decT[:, ec, :],
                        start=(ec == 0), stop=(ec == EC - 1),
                    )
                segT_sb = spool.tile([2, S], F32, tag="segT")
                nc.scalar.activation(segT_sb[:], seg_ps[:], AF.Identity, bias=sbias_sb[0:2, 1:2])
                nc.sync.dma_start(segT_d[i], segT_sb[:])
    if not nc.is_finalized():
        nc.finalize()
    return nc


def _logsumexp(a, axis):
    m = np.max(a, axis=axis, keepdims=True)
    out = np.log(np.sum(np.exp(a - m), axis=axis)) + np.squeeze(m, axis=axis)
    return out.astype(np.float32)


def _host_crf(em, labels, maskf, start_t, end_t, trans):
    b, s, t = em.shape
    bidx = np.arange(b)
    # numerator
    num = start_t[labels[:, 0]] + em[bidx, 0, labels[:, 0]]
    prev, cur = labels[:, :-1], labels[:, 1:]
    em_t = np.take_along_axis(em[:, 1:], cur[:, :, None], axis=2)[..., 0]
    num = num + ((trans[prev, cur] + em_t) * maskf[:, 1:]).sum(axis=1, dtype=np.float32)
    seq_ends = maskf.sum(axis=1).astype(np.int32) - 1
    num = num + end_t[labels[bidx, seq_ends]]
    # normalizer
    score = start_t[None] + em[:, 0]
    for ti in range(1, s):
        nxt = _logsumexp(score[:, :, None] + trans[None] + em[:, ti][:, None, :], axis=1)
        score = np.where(maskf[:, ti][:, None] > 0, nxt, score)
    den = _logsumexp(score + end_t[None], axis=1)
    # viterbi (no mask)
    vscore = start_t[None] + em[:, 0]
    history = np.empty((s - 1, b, t), np.int64)
    for ti in range(1, s):
        nxt = vscore[:, :, None] + trans[None] + em[:, ti][:, None, :]
        history[ti - 1] = np.argmax(nxt, axis=1)
        vscore = np.max(nxt, axis=1)
    last = np.argmax(vscore + end_t[None], axis=1)
    path = np.empty((s, b), np.int64)
    path[s - 1] = last
    tag = last
    for ti in range(s - 2, -1, -1):
        tag = history[ti][bidx, tag]
        path[ti] = tag
    return num.astype(np.float32), den, path.T


def kernel(**inputs):
    enc = np.ascontiguousarray(np.asarray(inputs["encoder_outputs"], np.float32))
    labels = np.asarray(inputs["labels"]).astype(np.int64)
    mask = np.asarray(inputs["mask"])
    Win = np.asarray(inputs["Win"], np.float32)
    bin_ = np.asarray(inputs["bin_"], np.float32)
    Wout = np.asarray(inputs["Wout"], np.float32)
    bout = np.asarray(inputs["bout"], np.float32)
    crf_w = np.asarray(inputs["crf_w"], np.float32)
    crf_b = np.asarray(inputs["crf_b"], np.float32)
    start_t = np.asarray(inputs["start_t"], np.float32)
    end_t = np.asarray(inputs["end_t"], np.float32)
    trans = np.asarray(inputs["trans"], np.float32)
    ent_w = np.asarray(inputs["ent_w"], np.float32)
    ent_b = np.asarray(inputs["ent_b"], np.float32)

    if "nc" not in _cached:
        _cached["nc"] = _build()
    nc = _cached["nc"]

    wpack = np.concatenate(
        [
            np.ascontiguousarray(Win.T).reshape(4, 128, 3 * E),
            np.ascontiguousarray(Wout.T).reshape(4, 128, E),
            np.ascontiguousarray(crf_w.T).reshape(4, 128, T),
            np.ascontiguousarray(ent_w.T).reshape(4, 128, 2),
        ],
        axis=2,
    )
    bpack = np.concatenate(
        [bin_.reshape(12, 128).T, bout.reshape(4, 128).T], axis=1
    )
    sbias = np.zeros((32, 2), np.float32)
    sbias[0:T, 0] = crf_b
    sbias[0:2, 1] = ent_b
    common = {
        "wpack": np.ascontiguousarray(wpack),
        "bpack": np.ascontiguousarray(bpack),
        "sbias": np.ascontiguousarray(sbias),
    }
    in_maps = [
        {"x": np.ascontiguousarray(enc[c * BS : (c + 1) * BS]), **common}
        for c in range(NC)
    ]
    res = run_bass_kernel_spmd(nc, in_maps, list(range(NC))).results

    fc = np.concatenate([r["fcT"] for r in res], axis=0).transpose(0, 2, 1)
    seg_logits = np.concatenate([r["segT"] for r in res], axis=0).transpose(0, 2, 1)
    fc = np.ascontiguousarray(fc, np.float32)

    maskf = mask.astype(np.float32)
    num, den, path = _host_crf(fc, labels, maskf, start_t, end_t, trans)
    llh = (num - den).sum(dtype=np.float32) / maskf.sum(dtype=np.float32)
    seg_out = (seg_logits - _logsumexp(seg_logits, axis=2)[:, :, None]).astype(np.float32)
    return path.astype(np.int32), seg_out, np.float32(-llh)
